# revision 23
# baseline (speedup 1.0000x reference)
"""Causal self-attention (B=4,T=2048,E=1024,H=16,D=64) on 8 trn2 cores.

Sharding: core = (batch b, head-group g) with g in {0,1} selecting 8 of 16
heads. Each core computes qkv projection for its batch restricted to its
head-group's columns, RoPE, causal attention for its 8 heads, and a partial
c_proj (its 512 rows of W_proj). Host sums the two partials per batch and
adds b_proj.

In-kernel layout choices:
 - x is passed pre-transposed (E, T) so all matmuls contract naturally.
 - q/k feature dims are permuted host-side into [all rope-even | all rope-odd]
   order (score-invariant since q and k share the permutation), making RoPE
   a handful of full-width vector ops.
 - scores are built transposed, S^T (k, q): softmax denominator comes from a
   ones column prepended to V (PV matmul row 0 = sum_k P), so no partition
   reductions and no P transpose are needed. Softmax skips max-subtraction
   (scores are O(10) here; exp is safe in fp32).
 - 1/l cannot fold through c_proj (it is per-head), so out^T is scaled per
   head before projection via a gpsimd partition broadcast of 1/l.
"""

import os
import numpy as np

import concourse.bass as bass
import concourse.bacc as bacc
import concourse.tile as tile
from concourse import mybir
from concourse.bass_utils import run_bass_kernel_spmd

B, T, E, H, D = 4, 2048, 1024, 16, 64
PHASES = int(os.environ.get('K_PHASES', '4'))
G = 8            # heads per core
F = G * D        # 512 features per core
THETA = 10000.0
NCORES = 8
KT = E // 128    # 8 contraction tiles for qkv
QC = 1024        # q chunk size in attention
NQC = T // QC    # 2

F32 = mybir.dt.float32
F32R = mybir.dt.float32r
AF = mybir.ActivationFunctionType
ALU = mybir.AluOpType


def r(ap):
    return ap.bitcast(F32R)


def build_nc():
    nc = bacc.Bacc(None)
    xT = nc.declare_dram_parameter("xT", [E, T], F32, isOutput=False)
    wq = nc.declare_dram_parameter("wq", [E, F], F32, isOutput=False)
    wk = nc.declare_dram_parameter("wk", [E, F], F32, isOutput=False)
    wv = nc.declare_dram_parameter("wv", [E, F], F32, isOutput=False)
    wp = nc.declare_dram_parameter("wp", [F, E], F32, isOutput=False)
    bqd = nc.declare_dram_parameter("bq", [1, F], F32, isOutput=False)
    bkd = nc.declare_dram_parameter("bk", [1, F], F32, isOutput=False)
    bvd = nc.declare_dram_parameter("bv", [1, F], F32, isOutput=False)
    cosd = nc.declare_dram_parameter("cosT", [128, T], F32, isOutput=False)
    sind = nc.declare_dram_parameter("sinT", [128, T], F32, isOutput=False)
    maskd = nc.declare_dram_parameter("maskadd", [128, 128], F32, isOutput=False)
    onesd = nc.declare_dram_parameter("ones", [128, 512], F32, isOutput=False)
    y = nc.declare_dram_parameter("y", [T, E], F32, isOutput=True)

    with tile.TileContext(nc) as tc:
        with tc.tile_pool(name="persist", bufs=1) as persist:
            mask_sb = persist.tile([128, 128], F32, tag="mask", name="mask")
            bq_sb = persist.tile([1, F], F32, tag="bq", name="bq")
            bk_sb = persist.tile([1, F], F32, tag="bk", name="bk")
            bv_sb = persist.tile([1, F], F32, tag="bv", name="bv")
            ones_sb = persist.tile([1, 512], F32, tag="ones", name="ones")
            outp = [persist.tile([128, T], F32, tag=f"outp{p}", name=f"outp{p}") for p in range(4)]

            nc.sync.dma_start(mask_sb[:], maskd[:])
            nc.sync.dma_start(r(bq_sb[:]), r(bqd[:]))
            nc.sync.dma_start(r(bk_sb[:]), r(bkd[:]))
            nc.sync.dma_start(r(bv_sb[:]), r(bvd[:]))
            nc.sync.dma_start(r(ones_sb[:]), r(onesd[0:1, :]))

            with tc.tile_pool(name="qkvpool", bufs=1) as qkvp:
                qt = [qkvp.tile([128, T], F32, tag=f"qt{i}", name=f"qt{i}") for i in range(4)]
                kt = [qkvp.tile([128, T], F32, tag=f"kt{i}", name=f"kt{i}") for i in range(4)]
                vp = [qkvp.tile([128, G, D + 1], F32, tag=f"vp{i}", name=f"vp{i}") for i in range(16)]
                for i in range(16):
                    nc.sync.dma_start(r(vp[i][:, :, 0:1]),
                                      r(onesd[:, 0:G].unsqueeze(2)))

                # ---- phase 1: qkv projection (x streamed in eighths) ----
                with (
                    tc.tile_pool(name="xload", bufs=2) as xpool,
                    tc.tile_pool(name="wload", bufs=1) as wpool,
                    tc.tile_pool(name="qkvps", bufs=4, space="PSUM") as qkv_ps,
                ):
                    for wdram, dst, brow in (
                        (wq, qt, bq_sb), (wk, kt, bk_sb), (wv, None, None)
                    ):
                        wts = []
                        for k in range(KT):
                            wt = wpool.tile([128, F], F32, tag=f"w{k}", name=f"w{k}")
                            nc.sync.dma_start(r(wt[:]), r(wdram[k * 128:(k + 1) * 128, :]))
                            wts.append(wt)
                        for qtr in range(4):
                            t0 = qtr * 512
                            xts = []
                            for k in range(KT):
                                xt = xpool.tile([128, 512], F32, tag=f"xt{k}", name=f"xt{k}")
                                nc.sync.dma_start(r(xt[:]), r(xT[k * 128:(k + 1) * 128, t0:t0 + 512]))
                                xts.append(xt)
                            if dst is not None:
                                # feature-major: out (feat, tok)
                                for f in range(4):
                                    ps = qkv_ps.tile([128, 512], F32, tag="qkv", name="qkv")
                                    for k in range(KT):
                                        nc.tensor.matmul(
                                            ps[:],
                                            r(wts[k][:, f * 128:(f + 1) * 128]),
                                            r(xts[k][:]),
                                            start=(k == 0), stop=False,
                                        )
                                    nc.tensor.matmul(
                                        ps[:], r(brow[0:1, f * 128:(f + 1) * 128]),
                                        r(ones_sb[0:1, 0:512]),
                                        start=False, stop=True,
                                    )
                                    nc.scalar.activation(
                                        r(dst[f][:, t0:t0 + 512]),
                                        ps[:], AF.Copy,
                                    )
                            else:
                                # token-major V, ones col at 0, bias via K=1 matmul
                                for tt in range(4):
                                    ti = qtr * 4 + tt
                                    ps = qkv_ps.tile([128, F], F32, tag="qkvv", name="qkvv")
                                    for k in range(KT):
                                        nc.tensor.matmul(
                                            ps[:],
                                            r(xts[k][:, tt * 128:(tt + 1) * 128]),
                                            r(wts[k][:]),
                                            start=(k == 0), stop=False,
                                        )
                                    nc.tensor.matmul(
                                        ps[:], r(ones_sb[0:1, 0:128]), r(bv_sb[:]),
                                        start=False, stop=True,
                                    )
                                    nc.scalar.activation(
                                        r(vp[ti][:, :, 1:]),
                                        ps[:].rearrange("p (h d) -> p h d", h=G),
                                        AF.Copy,
                                    )

                # ---- phase 2: rope (q on vector, k on gpsimd) ----
                with (
                    tc.tile_pool(name="trig", bufs=1) as trig,
                    tc.tile_pool(name="ropetmp", bufs=2) as rp,
                ):
                    cos_sb = trig.tile([128, T], F32, tag="cos", name="cos")
                    sin_sb = trig.tile([128, T], F32, tag="sin", name="sin")
                    nc.sync.dma_start(cos_sb[:], cosd[:])
                    nc.sync.dma_start(sin_sb[:], sind[:])
                    for src, eng0 in (((qt, nc.vector), (kt, nc.gpsimd)) if PHASES >= 2 else ()):
                        for g in range(2):
                            et, ot = src[g], src[2 + g]
                            for hf in range(4):
                                engine = eng0 if (src is qt or hf % 2 == 0) else nc.vector
                                s = slice(hf * 512, (hf + 1) * 512)
                                t1 = rp.tile([128, 512], F32, tag="t1", name="t1")
                                t2 = rp.tile([128, 512], F32, tag="t2", name="t2")
                                t3 = rp.tile([128, 512], F32, tag="t3", name="t3")
                                engine.tensor_tensor(t1[:], et[:, s], cos_sb[:, s], ALU.mult)
                                engine.tensor_tensor(t2[:], et[:, s], sin_sb[:, s], ALU.mult)
                                engine.tensor_tensor(t3[:], ot[:, s], sin_sb[:, s], ALU.mult)
                                engine.tensor_tensor(r(et[:, s]), t1[:], t3[:], ALU.subtract)
                                engine.tensor_tensor(t1[:], ot[:, s], cos_sb[:, s], ALU.mult)
                                engine.tensor_tensor(r(ot[:, s]), t1[:], t2[:], ALU.add)

                # ---- phase 3: attention, head pairs interleaved ----
                with (
                    tc.tile_pool(name="sps", bufs=1, space="PSUM") as spool,
                    tc.tile_pool(name="ops", bufs=1, space="PSUM") as opool,
                    tc.tile_pool(name="pexp", bufs=3) as ppool,
                    tc.tile_pool(name="stage", bufs=1) as stpool,
                ):
                    for pr in range(4 if PHASES >= 3 else 0):
                        heads = (2 * pr, 2 * pr + 1)
                        for c in range(NQC):
                            nkt = (c + 1) * QC // 128
                            ops_ = {}
                            for h in heads:
                                ops_[h] = opool.tile([D + 1, QC], F32, tag=f"o{h % 2}", name=f"o{h % 2}")
                            for k in range(nkt):
                                jj = k - c * (QC // 128)
                                q0 = 128 * jj if jj >= 0 else 0
                                for h in heads:
                                    row = (h % 4) * 32
                                    g = h // 4
                                    sps = spool.tile([128, QC], F32, tag=f"s{h % 2}", name=f"s{h % 2}")
                                    for a in (0, 512):
                                        lo = max(a, q0)
                                        if lo >= a + 512:
                                            continue
                                        nc.tensor.matmul(
                                            sps[:, lo:a + 512],
                                            r(kt[g][row:row + 32, k * 128:(k + 1) * 128]),
                                            r(qt[g][row:row + 32, c * QC + lo: c * QC + a + 512]),
                                            start=True, stop=False,
                                            tile_position=(row, 0),
                                            skip_group_check=True,
                                        )
                                        nc.tensor.matmul(
                                            sps[:, lo:a + 512],
                                            r(kt[2 + g][row:row + 32, k * 128:(k + 1) * 128]),
                                            r(qt[2 + g][row:row + 32, c * QC + lo: c * QC + a + 512]),
                                            start=False, stop=True,
                                            tile_position=(row, 0),
                                            skip_group_check=True,
                                        )
                                    if jj >= 0:
                                        nc.vector.tensor_tensor(
                                            sps[:, q0:q0 + 128], sps[:, q0:q0 + 128],
                                            mask_sb[:], ALU.add,
                                        )
                                    pt = ppool.tile([128, QC], F32, tag=f"p{h % 2}", name=f"p{h % 2}")
                                    nc.scalar.activation(r(pt[:, q0:]), sps[:, q0:], AF.Exp)
                                    for a in (0, 512):
                                        lo = max(a, q0)
                                        if lo >= a + 512:
                                            continue
                                        nc.tensor.matmul(
                                            ops_[h][:, lo:a + 512],
                                            r(vp[k][:, h, :]),
                                            r(pt[:, lo:a + 512]),
                                            start=(k == 0), stop=(k == nkt - 1),
                                            skip_group_check=True,
                                        )
                            bc = stpool.tile([128, QC], F32, tag="bc", name="bc")
                            for h in heads:
                                # psum row 0 = softmax denom; rows 1..64 = head out
                                st = stpool.tile([D + 1, QC], F32, tag=f"st{h % 2}", name=f"st{h % 2}")
                                nc.scalar.activation(r(st[:]), ops_[h][:], AF.Copy)
                                nc.vector.reciprocal(st[0:1, :], st[0:1, :])
                                nc.sync.dma_start(
                                    r(outp[pr][(h % 2) * 64:(h % 2) * 64 + 64, c * QC:(c + 1) * QC]),
                                    r(st[1:65, :]),
                                )
                                base = (h % 2) * 64
                                nc.sync.dma_start(bc[base:base + 1, :], st[0:1, :])
                                sz = 1
                                while sz < 64:
                                    nc.sync.dma_start(bc[base + sz:base + 2 * sz, :],
                                                      bc[base:base + sz, :])
                                    sz *= 2
                            nc.vector.tensor_tensor(
                                r(outp[pr][:, c * QC:(c + 1) * QC]),
                                outp[pr][:, c * QC:(c + 1) * QC], bc[:], ALU.mult)

            # ---- phase 4: normalize + c_proj ----
            with (
                tc.tile_pool(name="wppool", bufs=1) as wppool,
                tc.tile_pool(name="yps", bufs=4, space="PSUM") as ypool,
                tc.tile_pool(name="ysb", bufs=2) as ysb_pool,
            ):
                wp_sb = []
                for p in range(4):
                    w = wppool.tile([128, E], F32, tag=f"wp{p}", name=f"wp{p}")
                    nc.sync.dma_start(r(w[:]), r(wp[p * 128:(p + 1) * 128, :]))
                    wp_sb.append(w)
                for qt_i in range(16 if PHASES >= 4 else 0):
                    yps = ypool.tile([128, E], F32, tag="y", name="y")
                    for e in range(2):
                        for p in range(4):
                            nc.tensor.matmul(
                                yps[:, e * 512:(e + 1) * 512],
                                r(outp[p][:, qt_i * 128:(qt_i + 1) * 128]),
                                r(wp_sb[p][:, e * 512:(e + 1) * 512]),
                                start=(p == 0), stop=(p == 3),
                            )
                    ysb = ysb_pool.tile([128, E], F32, tag="ysb", name="ysb")
                    nc.scalar.activation(ysb[:], yps[:], AF.Copy)
                    nc.sync.dma_start(y[qt_i * 128:(qt_i + 1) * 128, :], ysb[:])

    nc.finalize()
    return nc


_NC = None


def _get_nc():
    global _NC
    if _NC is None:
        _NC = build_nc()
    return _NC


def _perm_eo():
    idx = np.zeros(F, np.int64)
    for p in range(F):
        a, j = (p // 32) % 8, p % 32
        idx[p] = a * 64 + 2 * j + (1 if p >= 256 else 0)
    return idx


def _host_prep(x, W_attn, b_attn, W_proj):
    perm = _perm_eo()
    scale = float(D) ** -0.5
    Wq, Wk, Wv = (np.asarray(W_attn[:, i * E:(i + 1) * E]) for i in range(3))
    bq, bk, bv = (np.asarray(b_attn[i * E:(i + 1) * E]) for i in range(3))

    pos = np.arange(T, dtype=np.float64)
    inv_freq = 1.0 / (THETA ** (np.arange(0, D, 2, dtype=np.float64) / D))
    freqs = np.outer(pos, inv_freq)          # (T, 32)
    cosT = np.tile(np.cos(freqs).T, (4, 1)).astype(np.float32)  # (128, T)
    sinT = np.tile(np.sin(freqs).T, (4, 1)).astype(np.float32)

    kp = np.arange(128)[:, None]
    qf = np.arange(128)[None, :]
    maskadd = np.where(qf >= kp, 0.0, -60.0).astype(np.float32)

    in_maps = []
    for core in range(NCORES):
        b, g = divmod(core, 2)
        cols = g * F + perm
        in_maps.append({
            "xT": np.ascontiguousarray(np.asarray(x[b]).T, np.float32),
            "wq": np.ascontiguousarray(Wq[:, cols] * scale, np.float32),
            "wk": np.ascontiguousarray(Wk[:, cols], np.float32),
            "wv": np.ascontiguousarray(Wv[:, g * F:(g + 1) * F], np.float32),
            "wp": np.ascontiguousarray(W_proj[g * F:(g + 1) * F, :], np.float32),
            "bq": (bq[None, cols] * scale).astype(np.float32),
            "bk": bk[None, cols].astype(np.float32),
            "bv": bv[None, g * F:(g + 1) * F].astype(np.float32),
            "ones": np.ones((128, 512), np.float32),
            "cosT": cosT,
            "sinT": sinT,
            "maskadd": maskadd,
        })
    return in_maps


def kernel(x, W_attn, b_attn, W_proj, b_proj):
    x = np.asarray(x, np.float32)
    W_attn = np.asarray(W_attn, np.float32)
    b_attn = np.asarray(b_attn, np.float32)
    W_proj = np.asarray(W_proj, np.float32)
    b_proj = np.asarray(b_proj, np.float32)

    nc = _get_nc()
    in_maps = _host_prep(x, W_attn, b_attn, W_proj)
    res = run_bass_kernel_spmd(nc, in_maps, list(range(NCORES)))
    parts = [res.results[c]["y"] for c in range(NCORES)]
    out = np.empty((B, T, E), np.float32)
    for b in range(B):
        out[b] = parts[2 * b] + parts[2 * b + 1] + b_proj[None, :]
    return out


# revision 25
# speedup vs baseline: 1.0440x; 1.0440x over previous
"""Causal self-attention (B=4,T=2048,E=1024,H=16,D=64) on 8 trn2 cores.

Sharding: core = (batch b, head-group g) with g in {0,1} selecting 8 of 16
heads. Each core computes qkv projection for its batch restricted to its
head-group's columns, RoPE, causal attention for its 8 heads, and a partial
c_proj (its 512 rows of W_proj). Host sums the two partials per batch and
adds b_proj.

In-kernel layout choices:
 - x is passed pre-transposed (E, T) so all matmuls contract naturally.
 - q/k feature dims are permuted host-side into [all rope-even | all rope-odd]
   order (score-invariant since q and k share the permutation), making RoPE
   a handful of full-width vector ops.
 - scores are built transposed, S^T (k, q): softmax denominator comes from a
   ones column prepended to V (PV matmul row 0 = sum_k P), so no partition
   reductions and no P transpose are needed. Softmax skips max-subtraction
   (scores are O(10) here; exp is safe in fp32).
 - 1/l cannot fold through c_proj (it is per-head), so out^T is scaled per
   head before projection via a gpsimd partition broadcast of 1/l.
"""

import os
import numpy as np

import concourse.bass as bass
import concourse.bacc as bacc
import concourse.tile as tile
from concourse import mybir
from concourse.bass_utils import run_bass_kernel_spmd

B, T, E, H, D = 4, 2048, 1024, 16, 64
PHASES = int(os.environ.get('K_PHASES', '4'))
G = 8            # heads per core
F = G * D        # 512 features per core
THETA = 10000.0
NCORES = 8
KT = E // 128    # 8 contraction tiles for qkv
QC = 1024        # q chunk size in attention
NQC = T // QC    # 2

F32 = mybir.dt.float32
F32R = mybir.dt.float32r
AF = mybir.ActivationFunctionType
ALU = mybir.AluOpType


def r(ap):
    return ap.bitcast(F32R)


def build_nc():
    nc = bacc.Bacc(None)
    xT = nc.declare_dram_parameter("xT", [E, T], F32, isOutput=False)
    wq = nc.declare_dram_parameter("wq", [E, F], F32, isOutput=False)
    wk = nc.declare_dram_parameter("wk", [E, F], F32, isOutput=False)
    wv = nc.declare_dram_parameter("wv", [E, F], F32, isOutput=False)
    wp = nc.declare_dram_parameter("wp", [F, E], F32, isOutput=False)
    bqd = nc.declare_dram_parameter("bq", [1, F], F32, isOutput=False)
    bkd = nc.declare_dram_parameter("bk", [1, F], F32, isOutput=False)
    bvd = nc.declare_dram_parameter("bv", [1, F], F32, isOutput=False)
    cosd = nc.declare_dram_parameter("cosT", [128, T], F32, isOutput=False)
    sind = nc.declare_dram_parameter("sinT", [128, T], F32, isOutput=False)
    maskd = nc.declare_dram_parameter("maskadd", [128, 128], F32, isOutput=False)
    onesd = nc.declare_dram_parameter("ones", [128, 512], F32, isOutput=False)
    y = nc.declare_dram_parameter("y", [T, E], F32, isOutput=True)

    with tile.TileContext(nc) as tc:
        with tc.tile_pool(name="persist", bufs=1) as persist:
            mask_sb = persist.tile([128, 128], F32, tag="mask", name="mask")
            bq_sb = persist.tile([1, F], F32, tag="bq", name="bq")
            bk_sb = persist.tile([1, F], F32, tag="bk", name="bk")
            bv_sb = persist.tile([1, F], F32, tag="bv", name="bv")
            ones_sb = persist.tile([1, 512], F32, tag="ones", name="ones")

            nc.sync.dma_start(mask_sb[:], maskd[:])
            nc.sync.dma_start(r(bq_sb[:]), r(bqd[:]))
            nc.sync.dma_start(r(bk_sb[:]), r(bkd[:]))
            nc.sync.dma_start(r(bv_sb[:]), r(bvd[:]))
            nc.sync.dma_start(r(ones_sb[:]), r(onesd[0:1, :]))

            with tc.tile_pool(name="qkvpool", bufs=1) as qkvp:
                qt = [qkvp.tile([128, T], F32, tag=f"qt{i}", name=f"qt{i}") for i in range(4)]
                kt = [qkvp.tile([128, T], F32, tag=f"kt{i}", name=f"kt{i}") for i in range(4)]
                vp = [qkvp.tile([128, G, D + 1], F32, tag=f"vp{i}", name=f"vp{i}") for i in range(16)]
                for i in range(16):
                    nc.sync.dma_start(r(vp[i][:, :, 0:1]),
                                      r(onesd[:, 0:G].unsqueeze(2)))

                # ---- phase 1: qkv projection (x streamed in eighths) ----
                with (
                    tc.tile_pool(name="xload", bufs=2) as xpool,
                    tc.tile_pool(name="wload", bufs=1) as wpool,
                    tc.tile_pool(name="qkvps", bufs=4, space="PSUM") as qkv_ps,
                ):
                    for wdram, dst, brow in (
                        (wq, qt, bq_sb), (wk, kt, bk_sb), (wv, None, None)
                    ):
                        wts = []
                        for k in range(KT):
                            wt = wpool.tile([128, F], F32, tag=f"w{k}", name=f"w{k}")
                            nc.sync.dma_start(r(wt[:]), r(wdram[k * 128:(k + 1) * 128, :]))
                            wts.append(wt)
                        for qtr in range(4):
                            t0 = qtr * 512
                            xts = []
                            for k in range(KT):
                                xt = xpool.tile([128, 512], F32, tag=f"xt{k}", name=f"xt{k}")
                                nc.sync.dma_start(r(xt[:]), r(xT[k * 128:(k + 1) * 128, t0:t0 + 512]))
                                xts.append(xt)
                            if dst is not None:
                                # feature-major: out (feat, tok)
                                for f in range(4):
                                    ps = qkv_ps.tile([128, 512], F32, tag="qkv", name="qkv")
                                    for k in range(KT):
                                        nc.tensor.matmul(
                                            ps[:],
                                            r(wts[k][:, f * 128:(f + 1) * 128]),
                                            r(xts[k][:]),
                                            start=(k == 0), stop=False,
                                        )
                                    nc.tensor.matmul(
                                        ps[:], r(brow[0:1, f * 128:(f + 1) * 128]),
                                        r(ones_sb[0:1, 0:512]),
                                        start=False, stop=True,
                                    )
                                    nc.scalar.activation(
                                        r(dst[f][:, t0:t0 + 512]),
                                        ps[:], AF.Copy,
                                    )
                            else:
                                # token-major V, ones col at 0, bias via K=1 matmul
                                for tt in range(4):
                                    ti = qtr * 4 + tt
                                    ps = qkv_ps.tile([128, F], F32, tag="qkvv", name="qkvv")
                                    for k in range(KT):
                                        nc.tensor.matmul(
                                            ps[:],
                                            r(xts[k][:, tt * 128:(tt + 1) * 128]),
                                            r(wts[k][:]),
                                            start=(k == 0), stop=False,
                                        )
                                    nc.tensor.matmul(
                                        ps[:], r(ones_sb[0:1, 0:128]), r(bv_sb[:]),
                                        start=False, stop=True,
                                    )
                                    nc.scalar.activation(
                                        r(vp[ti][:, :, 1:]),
                                        ps[:].rearrange("p (h d) -> p h d", h=G),
                                        AF.Copy,
                                    )

                # ---- phase 2: rope (q on vector, k on gpsimd) ----
                with (
                    tc.tile_pool(name="trig", bufs=1) as trig,
                    tc.tile_pool(name="ropetmp", bufs=2) as rp,
                ):
                    cos_sb = trig.tile([128, T], F32, tag="cos", name="cos")
                    sin_sb = trig.tile([128, T], F32, tag="sin", name="sin")
                    nc.sync.dma_start(cos_sb[:], cosd[:])
                    nc.sync.dma_start(sin_sb[:], sind[:])
                    for src, eng0 in (((qt, nc.vector), (kt, nc.gpsimd)) if PHASES >= 2 else ()):
                        for g in range(2):
                            et, ot = src[g], src[2 + g]
                            for hf in range(4):
                                engine = eng0 if (src is qt or hf % 2 == 0) else nc.vector
                                s = slice(hf * 512, (hf + 1) * 512)
                                t1 = rp.tile([128, 512], F32, tag="t1", name="t1")
                                t2 = rp.tile([128, 512], F32, tag="t2", name="t2")
                                t3 = rp.tile([128, 512], F32, tag="t3", name="t3")
                                engine.tensor_tensor(t1[:], et[:, s], cos_sb[:, s], ALU.mult)
                                engine.tensor_tensor(t2[:], et[:, s], sin_sb[:, s], ALU.mult)
                                engine.tensor_tensor(t3[:], ot[:, s], sin_sb[:, s], ALU.mult)
                                engine.tensor_tensor(r(et[:, s]), t1[:], t3[:], ALU.subtract)
                                engine.tensor_tensor(t1[:], ot[:, s], cos_sb[:, s], ALU.mult)
                                engine.tensor_tensor(r(ot[:, s]), t1[:], t2[:], ALU.add)

                # ---- phase 3: attention, head pairs interleaved ----
                outpool_cm = tc.tile_pool(name="outpool", bufs=1)
                outpool = outpool_cm.__enter__()
                outp = [outpool.tile([128, T], F32, tag=f"outp{p}", name=f"outp{p}") for p in range(4)]
                with (
                    tc.tile_pool(name="sps", bufs=1, space="PSUM") as spool,
                    tc.tile_pool(name="ops", bufs=1, space="PSUM") as opool,
                    tc.tile_pool(name="pexp", bufs=3) as ppool,
                    tc.tile_pool(name="stage", bufs=1) as stpool,
                ):
                    for pr in range(4 if PHASES >= 3 else 0):
                        heads = (2 * pr, 2 * pr + 1)
                        for c in range(NQC):
                            nkt = (c + 1) * QC // 128
                            ops_ = {}
                            for h in heads:
                                ops_[h] = opool.tile([D + 1, QC], F32, tag=f"o{h % 2}", name=f"o{h % 2}")
                            for k in range(nkt):
                                jj = k - c * (QC // 128)
                                q0 = 128 * jj if jj >= 0 else 0
                                for h in heads:
                                    row = (h % 4) * 32
                                    g = h // 4
                                    sps = spool.tile([128, QC], F32, tag=f"s{h % 2}", name=f"s{h % 2}")
                                    for a in (0, 512):
                                        lo = max(a, q0)
                                        if lo >= a + 512:
                                            continue
                                        nc.tensor.matmul(
                                            sps[:, lo:a + 512],
                                            r(kt[g][row:row + 32, k * 128:(k + 1) * 128]),
                                            r(qt[g][row:row + 32, c * QC + lo: c * QC + a + 512]),
                                            start=True, stop=False,
                                            tile_position=(row, 0),
                                            skip_group_check=True,
                                        )
                                        nc.tensor.matmul(
                                            sps[:, lo:a + 512],
                                            r(kt[2 + g][row:row + 32, k * 128:(k + 1) * 128]),
                                            r(qt[2 + g][row:row + 32, c * QC + lo: c * QC + a + 512]),
                                            start=False, stop=True,
                                            tile_position=(row, 0),
                                            skip_group_check=True,
                                        )
                                    pt = ppool.tile([128, QC], F32, tag=f"p{h % 2}", name=f"p{h % 2}")
                                    nc.scalar.activation(r(pt[:, q0:]), sps[:, q0:], AF.Exp)
                                    if jj >= 0:
                                        # zero the sub-diagonal triangle of the
                                        # diag block; only the narrow PV matmul
                                        # below waits on this.
                                        md = min(q0 + 128, QC)
                                        nc.vector.tensor_tensor(
                                            r(pt[:, q0:md]), pt[:, q0:md],
                                            mask_sb[:, 0:md - q0], ALU.mult,
                                        )
                                        segs = [(q0, md)]
                                        for a in (0, 512):
                                            lo = max(a, md)
                                            if lo < a + 512:
                                                segs.append((lo, a + 512))
                                    else:
                                        segs = [(0, 512), (512, QC)]
                                    for lo, hi in segs:
                                        nc.tensor.matmul(
                                            ops_[h][:, lo:hi],
                                            r(vp[k][:, h, :]),
                                            r(pt[:, lo:hi]),
                                            start=(k == 0), stop=(k == nkt - 1),
                                            skip_group_check=True,
                                        )
                            bc = stpool.tile([128, QC], F32, tag="bc", name="bc")
                            for h in heads:
                                # psum row 0 = softmax denom; rows 1..64 = head out
                                st = stpool.tile([D + 1, QC], F32, tag=f"st{h % 2}", name=f"st{h % 2}")
                                nc.scalar.activation(r(st[:]), ops_[h][:], AF.Copy)
                                nc.vector.reciprocal(st[0:1, :], st[0:1, :])
                                nc.sync.dma_start(
                                    r(outp[pr][(h % 2) * 64:(h % 2) * 64 + 64, c * QC:(c + 1) * QC]),
                                    r(st[1:65, :]),
                                )
                                base = (h % 2) * 64
                                nc.sync.dma_start(bc[base:base + 1, :], st[0:1, :])
                                sz = 1
                                while sz < 64:
                                    nc.sync.dma_start(bc[base + sz:base + 2 * sz, :],
                                                      bc[base:base + sz, :])
                                    sz *= 2
                            nc.vector.tensor_tensor(
                                r(outp[pr][:, c * QC:(c + 1) * QC]),
                                outp[pr][:, c * QC:(c + 1) * QC], bc[:], ALU.mult)

                # ---- phase 4: c_proj (inside qkvpool/outpool scope) ----
                with (
                    tc.tile_pool(name="wppool", bufs=1) as wppool,
                    tc.tile_pool(name="yps", bufs=4, space="PSUM") as ypool,
                    tc.tile_pool(name="ysb", bufs=2) as ysb_pool,
                ):
                    wp_sb = []
                    for p in range(4):
                        w = wppool.tile([128, E], F32, tag=f"wp{p}", name=f"wp{p}")
                        nc.sync.dma_start(r(w[:]), r(wp[p * 128:(p + 1) * 128, :]))
                        wp_sb.append(w)
                    for qt_i in range(16 if PHASES >= 4 else 0):
                        yps = ypool.tile([128, E], F32, tag="y", name="y")
                        for e in range(2):
                            for p in range(4):
                                nc.tensor.matmul(
                                    yps[:, e * 512:(e + 1) * 512],
                                    r(outp[p][:, qt_i * 128:(qt_i + 1) * 128]),
                                    r(wp_sb[p][:, e * 512:(e + 1) * 512]),
                                    start=(p == 0), stop=(p == 3),
                                )
                        ysb = ysb_pool.tile([128, E], F32, tag="ysb", name="ysb")
                        nc.scalar.activation(ysb[:], yps[:], AF.Copy)
                        nc.sync.dma_start(y[qt_i * 128:(qt_i + 1) * 128, :], ysb[:])
                outpool_cm.__exit__(None, None, None)

    nc.finalize()
    return nc


_NC = None


def _get_nc():
    global _NC
    if _NC is None:
        _NC = build_nc()
    return _NC


def _perm_eo():
    idx = np.zeros(F, np.int64)
    for p in range(F):
        a, j = (p // 32) % 8, p % 32
        idx[p] = a * 64 + 2 * j + (1 if p >= 256 else 0)
    return idx


def _host_prep(x, W_attn, b_attn, W_proj):
    perm = _perm_eo()
    scale = float(D) ** -0.5
    Wq, Wk, Wv = (np.asarray(W_attn[:, i * E:(i + 1) * E]) for i in range(3))
    bq, bk, bv = (np.asarray(b_attn[i * E:(i + 1) * E]) for i in range(3))

    pos = np.arange(T, dtype=np.float64)
    inv_freq = 1.0 / (THETA ** (np.arange(0, D, 2, dtype=np.float64) / D))
    freqs = np.outer(pos, inv_freq)          # (T, 32)
    cosT = np.tile(np.cos(freqs).T, (4, 1)).astype(np.float32)  # (128, T)
    sinT = np.tile(np.sin(freqs).T, (4, 1)).astype(np.float32)

    kp = np.arange(128)[:, None]
    qf = np.arange(128)[None, :]
    maskadd = np.where(qf >= kp, 1.0, 0.0).astype(np.float32)

    in_maps = []
    for core in range(NCORES):
        b, g = divmod(core, 2)
        cols = g * F + perm
        in_maps.append({
            "xT": np.ascontiguousarray(np.asarray(x[b]).T, np.float32),
            "wq": np.ascontiguousarray(Wq[:, cols] * scale, np.float32),
            "wk": np.ascontiguousarray(Wk[:, cols], np.float32),
            "wv": np.ascontiguousarray(Wv[:, g * F:(g + 1) * F], np.float32),
            "wp": np.ascontiguousarray(W_proj[g * F:(g + 1) * F, :], np.float32),
            "bq": (bq[None, cols] * scale).astype(np.float32),
            "bk": bk[None, cols].astype(np.float32),
            "bv": bv[None, g * F:(g + 1) * F].astype(np.float32),
            "ones": np.ones((128, 512), np.float32),
            "cosT": cosT,
            "sinT": sinT,
            "maskadd": maskadd,
        })
    return in_maps


def kernel(x, W_attn, b_attn, W_proj, b_proj):
    x = np.asarray(x, np.float32)
    W_attn = np.asarray(W_attn, np.float32)
    b_attn = np.asarray(b_attn, np.float32)
    W_proj = np.asarray(W_proj, np.float32)
    b_proj = np.asarray(b_proj, np.float32)

    nc = _get_nc()
    in_maps = _host_prep(x, W_attn, b_attn, W_proj)
    res = run_bass_kernel_spmd(nc, in_maps, list(range(NCORES)))
    parts = [res.results[c]["y"] for c in range(NCORES)]
    out = np.empty((B, T, E), np.float32)
    for b in range(B):
        out[b] = parts[2 * b] + parts[2 * b + 1] + b_proj[None, :]
    return out


# revision 26
# speedup vs baseline: 1.1314x; 1.0838x over previous
"""Causal self-attention (B=4,T=2048,E=1024,H=16,D=64) on 8 trn2 cores.

Sharding: core = (batch b, head-group g) with g in {0,1} selecting 8 of 16
heads. Each core computes qkv projection for its batch restricted to its
head-group's columns, RoPE, causal attention for its 8 heads, and a partial
c_proj (its 512 rows of W_proj). Host sums the two partials per batch and
adds b_proj.

In-kernel layout choices:
 - x is passed pre-transposed (E, T) so all matmuls contract naturally.
 - q/k feature dims are permuted host-side into [all rope-even | all rope-odd]
   order (score-invariant since q and k share the permutation), making RoPE
   a handful of full-width vector ops.
 - scores are built transposed, S^T (k, q): softmax denominator comes from a
   ones column prepended to V (PV matmul row 0 = sum_k P), so no partition
   reductions and no P transpose are needed. Softmax skips max-subtraction
   (scores are O(10) here; exp is safe in fp32).
 - 1/l cannot fold through c_proj (it is per-head), so out^T is scaled per
   head before projection via a gpsimd partition broadcast of 1/l.
"""

import os
import numpy as np

import concourse.bass as bass
import concourse.bacc as bacc
import concourse.tile as tile
from concourse import mybir
from concourse.bass_utils import run_bass_kernel_spmd

B, T, E, H, D = 4, 2048, 1024, 16, 64
PHASES = int(os.environ.get('K_PHASES', '4'))
G = 8            # heads per core
F = G * D        # 512 features per core
THETA = 10000.0
NCORES = 8
KT = E // 128    # 8 contraction tiles for qkv
QC = 1024        # q chunk size in attention
NQC = T // QC    # 2

F32 = mybir.dt.float32
F32R = mybir.dt.float32r
AF = mybir.ActivationFunctionType
ALU = mybir.AluOpType


def r(ap):
    return ap.bitcast(F32R)


def build_nc():
    nc = bacc.Bacc(None)
    xT = nc.declare_dram_parameter("xT", [E, T], F32, isOutput=False)
    wq = nc.declare_dram_parameter("wq", [E, F], F32, isOutput=False)
    wk = nc.declare_dram_parameter("wk", [E, F], F32, isOutput=False)
    wv = nc.declare_dram_parameter("wv", [E, F], F32, isOutput=False)
    wp = nc.declare_dram_parameter("wp", [F, E], F32, isOutput=False)
    bqd = nc.declare_dram_parameter("bq", [1, F], F32, isOutput=False)
    bkd = nc.declare_dram_parameter("bk", [1, F], F32, isOutput=False)
    bvd = nc.declare_dram_parameter("bv", [1, F], F32, isOutput=False)
    cosd = nc.declare_dram_parameter("cosT", [128, T], F32, isOutput=False)
    sind = nc.declare_dram_parameter("sinT", [128, T], F32, isOutput=False)
    maskd = nc.declare_dram_parameter("maskadd", [128, 128], F32, isOutput=False)
    onesd = nc.declare_dram_parameter("ones", [128, 512], F32, isOutput=False)
    y = nc.declare_dram_parameter("y", [T, E], F32, isOutput=True)

    with tile.TileContext(nc) as tc:
        with tc.tile_pool(name="persist", bufs=1) as persist:
            mask_sb = persist.tile([128, 128], F32, tag="mask", name="mask")
            bq_sb = persist.tile([1, F], F32, tag="bq", name="bq")
            bk_sb = persist.tile([1, F], F32, tag="bk", name="bk")
            bv_sb = persist.tile([1, F], F32, tag="bv", name="bv")
            ones_sb = persist.tile([1, 512], F32, tag="ones", name="ones")

            nc.sync.dma_start(mask_sb[:], maskd[:])
            nc.sync.dma_start(r(bq_sb[:]), r(bqd[:]))
            nc.sync.dma_start(r(bk_sb[:]), r(bkd[:]))
            nc.sync.dma_start(r(bv_sb[:]), r(bvd[:]))
            nc.sync.dma_start(r(ones_sb[:]), r(onesd[0:1, :]))

            with tc.tile_pool(name="qkvpool", bufs=1) as qkvp:
                qt = [qkvp.tile([128, T], F32, tag=f"qt{i}", name=f"qt{i}") for i in range(4)]
                kt = [qkvp.tile([128, T], F32, tag=f"kt{i}", name=f"kt{i}") for i in range(4)]
                vp = [qkvp.tile([128, G, D + 1], F32, tag=f"vp{i}", name=f"vp{i}") for i in range(16)]
                for i in range(16):
                    nc.sync.dma_start(r(vp[i][:, :, 0:1]),
                                      r(onesd[:, 0:G].unsqueeze(2)))

                # ---- phase 1: qkv projection + fused per-quarter rope ----
                def rope_quarter(dst, qtr, trig, rp):
                    t0 = qtr * 512
                    s = slice(t0, t0 + 512)
                    cos_q = trig.tile([128, 512], F32, tag="cosq", name="cosq")
                    sin_q = trig.tile([128, 512], F32, tag="sinq", name="sinq")
                    nc.sync.dma_start(cos_q[:], cosd[:, s])
                    nc.sync.dma_start(sin_q[:], sind[:, s])
                    for g in range(2):
                        engine = nc.vector if g == 0 else nc.gpsimd
                        et, ot = dst[g], dst[2 + g]
                        t1 = rp.tile([128, 512], F32, tag=f"t1{g}", name=f"t1{g}")
                        t2 = rp.tile([128, 512], F32, tag=f"t2{g}", name=f"t2{g}")
                        t3 = rp.tile([128, 512], F32, tag=f"t3{g}", name=f"t3{g}")
                        engine.tensor_tensor(t1[:], et[:, s], cos_q[:], ALU.mult)
                        engine.tensor_tensor(t2[:], et[:, s], sin_q[:], ALU.mult)
                        engine.tensor_tensor(t3[:], ot[:, s], sin_q[:], ALU.mult)
                        engine.tensor_tensor(r(et[:, s]), t1[:], t3[:], ALU.subtract)
                        engine.tensor_tensor(t1[:], ot[:, s], cos_q[:], ALU.mult)
                        engine.tensor_tensor(r(ot[:, s]), t1[:], t2[:], ALU.add)

                with (
                    tc.tile_pool(name="xload", bufs=2) as xpool,
                    tc.tile_pool(name="wload", bufs=1) as wpool,
                    tc.tile_pool(name="trig", bufs=2) as trig,
                    tc.tile_pool(name="ropetmp", bufs=2) as rp,
                    tc.tile_pool(name="qkvps", bufs=4, space="PSUM") as qkv_ps,
                ):
                    for wdram, dst, brow in (
                        (wq, qt, bq_sb), (wk, kt, bk_sb), (wv, None, None)
                    ):
                        wts = []
                        for k in range(KT):
                            wt = wpool.tile([128, F], F32, tag=f"w{k}", name=f"w{k}")
                            nc.sync.dma_start(r(wt[:]), r(wdram[k * 128:(k + 1) * 128, :]))
                            wts.append(wt)
                        for qtr in range(4):
                            t0 = qtr * 512
                            xts = []
                            for k in range(KT):
                                xt = xpool.tile([128, 512], F32, tag=f"xt{k}", name=f"xt{k}")
                                nc.sync.dma_start(r(xt[:]), r(xT[k * 128:(k + 1) * 128, t0:t0 + 512]))
                                xts.append(xt)
                            if dst is not None:
                                # feature-major: out (feat, tok)
                                for f in range(4):
                                    ps = qkv_ps.tile([128, 512], F32, tag="qkv", name="qkv")
                                    for k in range(KT):
                                        nc.tensor.matmul(
                                            ps[:],
                                            r(wts[k][:, f * 128:(f + 1) * 128]),
                                            r(xts[k][:]),
                                            start=(k == 0), stop=False,
                                        )
                                    nc.tensor.matmul(
                                        ps[:], r(brow[0:1, f * 128:(f + 1) * 128]),
                                        r(ones_sb[0:1, 0:512]),
                                        start=False, stop=True,
                                    )
                                    nc.scalar.activation(
                                        r(dst[f][:, t0:t0 + 512]),
                                        ps[:], AF.Copy,
                                    )
                                if PHASES >= 2:
                                    rope_quarter(dst, qtr, trig, rp)
                            else:
                                # token-major V, ones col at 0, bias via K=1 matmul
                                for tt in range(4):
                                    ti = qtr * 4 + tt
                                    ps = qkv_ps.tile([128, F], F32, tag="qkvv", name="qkvv")
                                    for k in range(KT):
                                        nc.tensor.matmul(
                                            ps[:],
                                            r(xts[k][:, tt * 128:(tt + 1) * 128]),
                                            r(wts[k][:]),
                                            start=(k == 0), stop=False,
                                        )
                                    nc.tensor.matmul(
                                        ps[:], r(ones_sb[0:1, 0:128]), r(bv_sb[:]),
                                        start=False, stop=True,
                                    )
                                    nc.scalar.activation(
                                        r(vp[ti][:, :, 1:]),
                                        ps[:].rearrange("p (h d) -> p h d", h=G),
                                        AF.Copy,
                                    )

                # ---- phase 3: attention, head pairs interleaved ----
                outpool_cm = tc.tile_pool(name="outpool", bufs=1)
                outpool = outpool_cm.__enter__()
                outp = [outpool.tile([128, T], F32, tag=f"outp{p}", name=f"outp{p}") for p in range(4)]
                with (
                    tc.tile_pool(name="sps", bufs=1, space="PSUM") as spool,
                    tc.tile_pool(name="ops", bufs=1, space="PSUM") as opool,
                    tc.tile_pool(name="pexp", bufs=3) as ppool,
                    tc.tile_pool(name="stage", bufs=1) as stpool,
                ):
                    for pr in range(4 if PHASES >= 3 else 0):
                        heads = (2 * pr, 2 * pr + 1)
                        for c in range(NQC):
                            nkt = (c + 1) * QC // 128
                            ops_ = {}
                            for h in heads:
                                ops_[h] = opool.tile([D + 1, QC], F32, tag=f"o{h % 2}", name=f"o{h % 2}")
                            for k in range(nkt):
                                jj = k - c * (QC // 128)
                                q0 = 128 * jj if jj >= 0 else 0
                                for h in heads:
                                    row = (h % 4) * 32
                                    g = h // 4
                                    sps = spool.tile([128, QC], F32, tag=f"s{h % 2}", name=f"s{h % 2}")
                                    for a in (0, 512):
                                        lo = max(a, q0)
                                        if lo >= a + 512:
                                            continue
                                        nc.tensor.matmul(
                                            sps[:, lo:a + 512],
                                            r(kt[g][row:row + 32, k * 128:(k + 1) * 128]),
                                            r(qt[g][row:row + 32, c * QC + lo: c * QC + a + 512]),
                                            start=True, stop=False,
                                            tile_position=(row, 0),
                                            skip_group_check=True,
                                        )
                                        nc.tensor.matmul(
                                            sps[:, lo:a + 512],
                                            r(kt[2 + g][row:row + 32, k * 128:(k + 1) * 128]),
                                            r(qt[2 + g][row:row + 32, c * QC + lo: c * QC + a + 512]),
                                            start=False, stop=True,
                                            tile_position=(row, 0),
                                            skip_group_check=True,
                                        )
                                    pt = ppool.tile([128, QC], F32, tag=f"p{h % 2}", name=f"p{h % 2}")
                                    nc.scalar.activation(r(pt[:, q0:]), sps[:, q0:], AF.Exp)
                                    if jj >= 0:
                                        # zero the sub-diagonal triangle of the
                                        # diag block; only the narrow PV matmul
                                        # below waits on this.
                                        md = min(q0 + 128, QC)
                                        nc.vector.tensor_tensor(
                                            r(pt[:, q0:md]), pt[:, q0:md],
                                            mask_sb[:, 0:md - q0], ALU.mult,
                                        )
                                        segs = [(q0, md)]
                                        for a in (0, 512):
                                            lo = max(a, md)
                                            if lo < a + 512:
                                                segs.append((lo, a + 512))
                                    else:
                                        segs = [(0, 512), (512, QC)]
                                    for lo, hi in segs:
                                        nc.tensor.matmul(
                                            ops_[h][:, lo:hi],
                                            r(vp[k][:, h, :]),
                                            r(pt[:, lo:hi]),
                                            start=(k == 0), stop=(k == nkt - 1),
                                            skip_group_check=True,
                                        )
                            bc = stpool.tile([128, QC], F32, tag="bc", name="bc")
                            for h in heads:
                                # psum row 0 = softmax denom; rows 1..64 = head out
                                st = stpool.tile([D + 1, QC], F32, tag=f"st{h % 2}", name=f"st{h % 2}")
                                nc.scalar.activation(r(st[:]), ops_[h][:], AF.Copy)
                                nc.vector.reciprocal(st[0:1, :], st[0:1, :])
                                nc.sync.dma_start(
                                    r(outp[pr][(h % 2) * 64:(h % 2) * 64 + 64, c * QC:(c + 1) * QC]),
                                    r(st[1:65, :]),
                                )
                                base = (h % 2) * 64
                                nc.sync.dma_start(bc[base:base + 1, :], st[0:1, :])
                                sz = 1
                                while sz < 64:
                                    nc.sync.dma_start(bc[base + sz:base + 2 * sz, :],
                                                      bc[base:base + sz, :])
                                    sz *= 2
                            nc.vector.tensor_tensor(
                                r(outp[pr][:, c * QC:(c + 1) * QC]),
                                outp[pr][:, c * QC:(c + 1) * QC], bc[:], ALU.mult)

                # ---- phase 4: c_proj (inside qkvpool/outpool scope) ----
                with (
                    tc.tile_pool(name="wppool", bufs=1) as wppool,
                    tc.tile_pool(name="yps", bufs=4, space="PSUM") as ypool,
                    tc.tile_pool(name="ysb", bufs=2) as ysb_pool,
                ):
                    wp_sb = []
                    for p in range(4):
                        w = wppool.tile([128, E], F32, tag=f"wp{p}", name=f"wp{p}")
                        nc.sync.dma_start(r(w[:]), r(wp[p * 128:(p + 1) * 128, :]))
                        wp_sb.append(w)
                    for qt_i in range(16 if PHASES >= 4 else 0):
                        yps = ypool.tile([128, E], F32, tag="y", name="y")
                        for e in range(2):
                            for p in range(4):
                                nc.tensor.matmul(
                                    yps[:, e * 512:(e + 1) * 512],
                                    r(outp[p][:, qt_i * 128:(qt_i + 1) * 128]),
                                    r(wp_sb[p][:, e * 512:(e + 1) * 512]),
                                    start=(p == 0), stop=(p == 3),
                                )
                        ysb = ysb_pool.tile([128, E], F32, tag="ysb", name="ysb")
                        nc.scalar.activation(ysb[:], yps[:], AF.Copy)
                        nc.sync.dma_start(y[qt_i * 128:(qt_i + 1) * 128, :], ysb[:])
                outpool_cm.__exit__(None, None, None)

    nc.finalize()
    return nc


_NC = None


def _get_nc():
    global _NC
    if _NC is None:
        _NC = build_nc()
    return _NC


def _perm_eo():
    idx = np.zeros(F, np.int64)
    for p in range(F):
        a, j = (p // 32) % 8, p % 32
        idx[p] = a * 64 + 2 * j + (1 if p >= 256 else 0)
    return idx


def _host_prep(x, W_attn, b_attn, W_proj):
    perm = _perm_eo()
    scale = float(D) ** -0.5
    Wq, Wk, Wv = (np.asarray(W_attn[:, i * E:(i + 1) * E]) for i in range(3))
    bq, bk, bv = (np.asarray(b_attn[i * E:(i + 1) * E]) for i in range(3))

    pos = np.arange(T, dtype=np.float64)
    inv_freq = 1.0 / (THETA ** (np.arange(0, D, 2, dtype=np.float64) / D))
    freqs = np.outer(pos, inv_freq)          # (T, 32)
    cosT = np.tile(np.cos(freqs).T, (4, 1)).astype(np.float32)  # (128, T)
    sinT = np.tile(np.sin(freqs).T, (4, 1)).astype(np.float32)

    kp = np.arange(128)[:, None]
    qf = np.arange(128)[None, :]
    maskadd = np.where(qf >= kp, 1.0, 0.0).astype(np.float32)

    in_maps = []
    for core in range(NCORES):
        b, g = divmod(core, 2)
        cols = g * F + perm
        in_maps.append({
            "xT": np.ascontiguousarray(np.asarray(x[b]).T, np.float32),
            "wq": np.ascontiguousarray(Wq[:, cols] * scale, np.float32),
            "wk": np.ascontiguousarray(Wk[:, cols], np.float32),
            "wv": np.ascontiguousarray(Wv[:, g * F:(g + 1) * F], np.float32),
            "wp": np.ascontiguousarray(W_proj[g * F:(g + 1) * F, :], np.float32),
            "bq": (bq[None, cols] * scale).astype(np.float32),
            "bk": bk[None, cols].astype(np.float32),
            "bv": bv[None, g * F:(g + 1) * F].astype(np.float32),
            "ones": np.ones((128, 512), np.float32),
            "cosT": cosT,
            "sinT": sinT,
            "maskadd": maskadd,
        })
    return in_maps


def kernel(x, W_attn, b_attn, W_proj, b_proj):
    x = np.asarray(x, np.float32)
    W_attn = np.asarray(W_attn, np.float32)
    b_attn = np.asarray(b_attn, np.float32)
    W_proj = np.asarray(W_proj, np.float32)
    b_proj = np.asarray(b_proj, np.float32)

    nc = _get_nc()
    in_maps = _host_prep(x, W_attn, b_attn, W_proj)
    res = run_bass_kernel_spmd(nc, in_maps, list(range(NCORES)))
    parts = [res.results[c]["y"] for c in range(NCORES)]
    out = np.empty((B, T, E), np.float32)
    for b in range(B):
        out[b] = parts[2 * b] + parts[2 * b + 1] + b_proj[None, :]
    return out


# revision 30
# speedup vs baseline: 1.1590x; 1.0244x over previous
"""Causal self-attention (B=4,T=2048,E=1024,H=16,D=64) on 8 trn2 cores.

Sharding: core = (batch b, head-group g) with g in {0,1} selecting 8 of 16
heads. Each core computes qkv projection for its batch restricted to its
head-group's columns, RoPE, causal attention for its 8 heads, and a partial
c_proj (its 512 rows of W_proj). Host sums the two partials per batch and
adds b_proj.

In-kernel layout choices:
 - x is passed pre-transposed (E, T) so all matmuls contract naturally.
 - q/k feature dims are permuted host-side into [all rope-even | all rope-odd]
   order (score-invariant since q and k share the permutation), making RoPE
   a handful of full-width vector ops.
 - scores are built transposed, S^T (k, q): softmax denominator comes from a
   ones column prepended to V (PV matmul row 0 = sum_k P), so no partition
   reductions and no P transpose are needed. Softmax skips max-subtraction
   (scores are O(10) here; exp is safe in fp32).
 - 1/l cannot fold through c_proj (it is per-head), so out^T is scaled per
   head before projection via a gpsimd partition broadcast of 1/l.
"""

import os
import numpy as np

import concourse.bass as bass
import concourse.bacc as bacc
import concourse.tile as tile
from concourse import mybir
from concourse.bass_utils import run_bass_kernel_spmd

B, T, E, H, D = 4, 2048, 1024, 16, 64
PHASES = int(os.environ.get('K_PHASES', '4'))
G = 8            # heads per core
F = G * D        # 512 features per core
THETA = 10000.0
NCORES = 8
KT = E // 128    # 8 contraction tiles for qkv
QC = 1024        # q chunk size in attention
NQC = T // QC    # 2

F32 = mybir.dt.float32
F32R = mybir.dt.float32r
AF = mybir.ActivationFunctionType
ALU = mybir.AluOpType


def r(ap):
    return ap.bitcast(F32R)


def build_nc():
    nc = bacc.Bacc(None)
    xT = nc.declare_dram_parameter("xT", [E, T], F32, isOutput=False)
    wq = nc.declare_dram_parameter("wq", [E, F], F32, isOutput=False)
    wk = nc.declare_dram_parameter("wk", [E, F], F32, isOutput=False)
    wv = nc.declare_dram_parameter("wv", [E, F], F32, isOutput=False)
    wp = nc.declare_dram_parameter("wp", [F, E], F32, isOutput=False)
    bqd = nc.declare_dram_parameter("bq", [1, F], F32, isOutput=False)
    bkd = nc.declare_dram_parameter("bk", [1, F], F32, isOutput=False)
    bvd = nc.declare_dram_parameter("bv", [1, F], F32, isOutput=False)
    cosd = nc.declare_dram_parameter("cosT", [128, T], F32, isOutput=False)
    sind = nc.declare_dram_parameter("sinT", [128, T], F32, isOutput=False)
    maskd = nc.declare_dram_parameter("maskadd", [128, 128], F32, isOutput=False)
    onesd = nc.declare_dram_parameter("ones", [128, 512], F32, isOutput=False)
    y = nc.declare_dram_parameter("y", [T, E], F32, isOutput=True)

    with tile.TileContext(nc) as tc:
        with tc.tile_pool(name="persist", bufs=1) as persist:
            mask_sb = persist.tile([128, 128], F32, tag="mask", name="mask")
            bq_sb = persist.tile([1, F], F32, tag="bq", name="bq")
            bk_sb = persist.tile([1, F], F32, tag="bk", name="bk")
            bv_sb = persist.tile([1, F], F32, tag="bv", name="bv")
            ones_sb = persist.tile([1, 512], F32, tag="ones", name="ones")

            nc.sync.dma_start(mask_sb[:], maskd[:])
            nc.sync.dma_start(r(bq_sb[:]), r(bqd[:]))
            nc.sync.dma_start(r(bk_sb[:]), r(bkd[:]))
            nc.sync.dma_start(r(bv_sb[:]), r(bvd[:]))
            nc.sync.dma_start(r(ones_sb[:]), r(onesd[0:1, :]))

            with tc.tile_pool(name="qkvpool", bufs=1) as qkvp:
                qt = [qkvp.tile([128, T], F32, tag=f"qt{i}", name=f"qt{i}") for i in range(4)]
                kt = [qkvp.tile([128, T], F32, tag=f"kt{i}", name=f"kt{i}") for i in range(4)]
                vp = [qkvp.tile([128, G, D + 1], F32, tag=f"vp{i}", name=f"vp{i}") for i in range(16)]
                for i in range(16):
                    nc.sync.dma_start(r(vp[i][:, :, 0:1]),
                                      r(onesd[:, 0:G].unsqueeze(2)))

                # ---- phase 1: qkv projection + fused per-quarter rope ----
                def rope_quarter(dst, qtr, trig, rp):
                    t0 = qtr * 512
                    s = slice(t0, t0 + 512)
                    cos_q = trig.tile([128, 512], F32, tag="cosq", name="cosq")
                    sin_q = trig.tile([128, 512], F32, tag="sinq", name="sinq")
                    nc.sync.dma_start(cos_q[:], cosd[:, s])
                    nc.sync.dma_start(sin_q[:], sind[:, s])
                    for g in range(2):
                        engine = nc.vector if g == 0 else nc.gpsimd
                        et, ot = dst[g], dst[2 + g]
                        t1 = rp.tile([128, 512], F32, tag=f"t1{g}", name=f"t1{g}")
                        t2 = rp.tile([128, 512], F32, tag=f"t2{g}", name=f"t2{g}")
                        t3 = rp.tile([128, 512], F32, tag=f"t3{g}", name=f"t3{g}")
                        engine.tensor_tensor(t1[:], et[:, s], cos_q[:], ALU.mult)
                        engine.tensor_tensor(t2[:], et[:, s], sin_q[:], ALU.mult)
                        engine.tensor_tensor(t3[:], ot[:, s], sin_q[:], ALU.mult)
                        engine.tensor_tensor(r(et[:, s]), t1[:], t3[:], ALU.subtract)
                        engine.tensor_tensor(t1[:], ot[:, s], cos_q[:], ALU.mult)
                        engine.tensor_tensor(r(ot[:, s]), t1[:], t2[:], ALU.add)

                with (
                    tc.tile_pool(name="xload", bufs=2) as xpool,
                    tc.tile_pool(name="wload", bufs=1) as wpool,
                    tc.tile_pool(name="trig", bufs=2) as trig,
                    tc.tile_pool(name="ropetmp", bufs=2) as rp,
                    tc.tile_pool(name="qkvps", bufs=4, space="PSUM") as qkv_ps,
                ):
                    for wdram, dst, brow in (
                        (wq, qt, bq_sb), (wk, kt, bk_sb), (wv, None, None)
                    ):
                        wts = []
                        for k in range(KT):
                            wt = wpool.tile([128, F], F32, tag=f"w{k}", name=f"w{k}")
                            nc.sync.dma_start(r(wt[:]), r(wdram[k * 128:(k + 1) * 128, :]))
                            wts.append(wt)
                        for qtr in range(4):
                            t0 = qtr * 512
                            xts = []
                            for k in range(KT):
                                xt = xpool.tile([128, 512], F32, tag=f"xt{k}", name=f"xt{k}")
                                nc.sync.dma_start(r(xt[:]), r(xT[k * 128:(k + 1) * 128, t0:t0 + 512]))
                                xts.append(xt)
                            if dst is not None:
                                # feature-major: out (feat, tok)
                                for f in range(4):
                                    ps = qkv_ps.tile([128, 512], F32, tag="qkv", name="qkv")
                                    for k in range(KT):
                                        nc.tensor.matmul(
                                            ps[:],
                                            r(wts[k][:, f * 128:(f + 1) * 128]),
                                            r(xts[k][:]),
                                            start=(k == 0), stop=False,
                                        )
                                    nc.tensor.matmul(
                                        ps[:], r(brow[0:1, f * 128:(f + 1) * 128]),
                                        r(ones_sb[0:1, 0:512]),
                                        start=False, stop=True,
                                    )
                                    nc.scalar.activation(
                                        r(dst[f][:, t0:t0 + 512]),
                                        ps[:], AF.Copy,
                                    )
                                if PHASES >= 2:
                                    rope_quarter(dst, qtr, trig, rp)
                            else:
                                # token-major V, ones col at 0, bias via K=1 matmul
                                for tt in range(4):
                                    ti = qtr * 4 + tt
                                    ps = qkv_ps.tile([128, F], F32, tag="qkvv", name="qkvv")
                                    for k in range(KT):
                                        nc.tensor.matmul(
                                            ps[:],
                                            r(xts[k][:, tt * 128:(tt + 1) * 128]),
                                            r(wts[k][:]),
                                            start=(k == 0), stop=False,
                                        )
                                    nc.tensor.matmul(
                                        ps[:], r(ones_sb[0:1, 0:128]), r(bv_sb[:]),
                                        start=False, stop=True,
                                    )
                                    nc.scalar.activation(
                                        r(vp[ti][:, :, 1:]),
                                        ps[:].rearrange("p (h d) -> p h d", h=G),
                                        AF.Copy,
                                    )

                # ---- phase 3: attention, head pairs interleaved ----
                outpool_cm = tc.tile_pool(name="outpool", bufs=1)
                outpool = outpool_cm.__enter__()
                outp = [outpool.tile([128, T], F32, tag=f"outp{p}", name=f"outp{p}") for p in range(4)]
                wp_sb = []
                for p in range(4):
                    w = outpool.tile([128, E], F32, tag=f"wp{p}", name=f"wp{p}")
                    nc.sync.dma_start(r(w[:]), r(wp[p * 128:(p + 1) * 128, :]))
                    wp_sb.append(w)
                with (
                    tc.tile_pool(name="sps", bufs=1, space="PSUM") as spool,
                    tc.tile_pool(name="ops", bufs=1, space="PSUM") as opool,
                    tc.tile_pool(name="pexp", bufs=3) as ppool,
                    tc.tile_pool(name="stage", bufs=1) as stpool,
                ):
                    for pr in range(4 if PHASES >= 3 else 0):
                        heads = (2 * pr, 2 * pr + 1)
                        for c in range(NQC):
                            nkt = (c + 1) * QC // 128
                            ops_ = {}
                            for h in heads:
                                ops_[h] = opool.tile([D + 1, QC], F32, tag=f"o{h % 2}", name=f"o{h % 2}")
                            for k in range(nkt):
                                jj = k - c * (QC // 128)
                                q0 = 128 * jj if jj >= 0 else 0
                                for h in heads:
                                    row = (h % 4) * 32
                                    g = h // 4
                                    sps = spool.tile([128, QC], F32, tag=f"s{h % 2}", name=f"s{h % 2}")
                                    for a in (0, 512):
                                        lo = max(a, q0)
                                        if lo >= a + 512:
                                            continue
                                        nc.tensor.matmul(
                                            sps[:, lo:a + 512],
                                            r(kt[g][row:row + 32, k * 128:(k + 1) * 128]),
                                            r(qt[g][row:row + 32, c * QC + lo: c * QC + a + 512]),
                                            start=True, stop=False,
                                            tile_position=(row, 0),
                                            skip_group_check=True,
                                        )
                                        nc.tensor.matmul(
                                            sps[:, lo:a + 512],
                                            r(kt[2 + g][row:row + 32, k * 128:(k + 1) * 128]),
                                            r(qt[2 + g][row:row + 32, c * QC + lo: c * QC + a + 512]),
                                            start=False, stop=True,
                                            tile_position=(row, 0),
                                            skip_group_check=True,
                                        )
                                    pt = ppool.tile([128, QC], F32, tag=f"p{h % 2}", name=f"p{h % 2}")
                                    nc.scalar.activation(r(pt[:, q0:]), sps[:, q0:], AF.Exp)
                                    if jj >= 0:
                                        # zero the sub-diagonal triangle of the
                                        # diag block; only the narrow PV matmul
                                        # below waits on this.
                                        md = min(q0 + 128, QC)
                                        nc.vector.tensor_tensor(
                                            r(pt[:, q0:md]), pt[:, q0:md],
                                            mask_sb[:, 0:md - q0], ALU.mult,
                                        )
                                        segs = [(q0, md)]
                                        for a in (0, 512):
                                            lo = max(a, md)
                                            if lo < a + 512:
                                                segs.append((lo, a + 512))
                                    else:
                                        segs = [(0, 512), (512, QC)]
                                    for lo, hi in segs:
                                        nc.tensor.matmul(
                                            ops_[h][:, lo:hi],
                                            r(vp[k][:, h, :]),
                                            r(pt[:, lo:hi]),
                                            start=(k == 0), stop=(k == nkt - 1),
                                            skip_group_check=True,
                                        )
                            bc = stpool.tile([128, QC], F32, tag="bc", name="bc")
                            for h in heads:
                                # psum row 0 = softmax denom; rows 1..64 = head out
                                st = stpool.tile([D + 1, QC], F32, tag=f"st{h % 2}", name=f"st{h % 2}")
                                nc.scalar.activation(r(st[:]), ops_[h][:], AF.Copy)
                                nc.vector.reciprocal(st[0:1, :], st[0:1, :])
                                nc.sync.dma_start(
                                    r(outp[pr][(h % 2) * 64:(h % 2) * 64 + 64, c * QC:(c + 1) * QC]),
                                    r(st[1:65, :]),
                                )
                                base = (h % 2) * 64
                                nc.sync.dma_start(bc[base:base + 1, :], st[0:1, :])
                                sz = 1
                                while sz < 64:
                                    nc.sync.dma_start(bc[base + sz:base + 2 * sz, :],
                                                      bc[base:base + sz, :])
                                    sz *= 2
                            nc.vector.tensor_tensor(
                                r(outp[pr][:, c * QC:(c + 1) * QC]),
                                outp[pr][:, c * QC:(c + 1) * QC], bc[:], ALU.mult)

                # ---- phase 4: c_proj (inside qkvpool/outpool scope) ----
                with (
                    tc.tile_pool(name="yps", bufs=4, space="PSUM") as ypool,
                    tc.tile_pool(name="ysb", bufs=2) as ysb_pool,
                ):
                    for qt_i in range(16 if PHASES >= 4 else 0):
                        yps = ypool.tile([128, E], F32, tag="y", name="y")
                        for e in range(2):
                            for p in range(4):
                                nc.tensor.matmul(
                                    yps[:, e * 512:(e + 1) * 512],
                                    r(outp[p][:, qt_i * 128:(qt_i + 1) * 128]),
                                    r(wp_sb[p][:, e * 512:(e + 1) * 512]),
                                    start=(p == 0), stop=(p == 3),
                                )
                        ysb = ysb_pool.tile([128, E], F32, tag="ysb", name="ysb")
                        nc.scalar.activation(ysb[:], yps[:], AF.Copy)
                        nc.sync.dma_start(y[qt_i * 128:(qt_i + 1) * 128, :], ysb[:])
                outpool_cm.__exit__(None, None, None)

    nc.finalize()
    return nc


_NC = None


def _get_nc():
    global _NC
    if _NC is None:
        _NC = build_nc()
    return _NC


def _perm_eo():
    idx = np.zeros(F, np.int64)
    for p in range(F):
        a, j = (p // 32) % 8, p % 32
        idx[p] = a * 64 + 2 * j + (1 if p >= 256 else 0)
    return idx


def _host_prep(x, W_attn, b_attn, W_proj):
    perm = _perm_eo()
    scale = float(D) ** -0.5
    Wq, Wk, Wv = (np.asarray(W_attn[:, i * E:(i + 1) * E]) for i in range(3))
    bq, bk, bv = (np.asarray(b_attn[i * E:(i + 1) * E]) for i in range(3))

    pos = np.arange(T, dtype=np.float64)
    inv_freq = 1.0 / (THETA ** (np.arange(0, D, 2, dtype=np.float64) / D))
    freqs = np.outer(pos, inv_freq)          # (T, 32)
    cosT = np.tile(np.cos(freqs).T, (4, 1)).astype(np.float32)  # (128, T)
    sinT = np.tile(np.sin(freqs).T, (4, 1)).astype(np.float32)

    kp = np.arange(128)[:, None]
    qf = np.arange(128)[None, :]
    maskadd = np.where(qf >= kp, 1.0, 0.0).astype(np.float32)

    in_maps = []
    for core in range(NCORES):
        b, g = divmod(core, 2)
        cols = g * F + perm
        in_maps.append({
            "xT": np.ascontiguousarray(np.asarray(x[b]).T, np.float32),
            "wq": np.ascontiguousarray(Wq[:, cols] * scale, np.float32),
            "wk": np.ascontiguousarray(Wk[:, cols], np.float32),
            "wv": np.ascontiguousarray(Wv[:, g * F:(g + 1) * F], np.float32),
            "wp": np.ascontiguousarray(W_proj[g * F:(g + 1) * F, :], np.float32),
            "bq": (bq[None, cols] * scale).astype(np.float32),
            "bk": bk[None, cols].astype(np.float32),
            "bv": bv[None, g * F:(g + 1) * F].astype(np.float32),
            "ones": np.ones((128, 512), np.float32),
            "cosT": cosT,
            "sinT": sinT,
            "maskadd": maskadd,
        })
    return in_maps


def kernel(x, W_attn, b_attn, W_proj, b_proj):
    x = np.asarray(x, np.float32)
    W_attn = np.asarray(W_attn, np.float32)
    b_attn = np.asarray(b_attn, np.float32)
    W_proj = np.asarray(W_proj, np.float32)
    b_proj = np.asarray(b_proj, np.float32)

    nc = _get_nc()
    in_maps = _host_prep(x, W_attn, b_attn, W_proj)
    res = run_bass_kernel_spmd(nc, in_maps, list(range(NCORES)))
    parts = [res.results[c]["y"] for c in range(NCORES)]
    out = np.empty((B, T, E), np.float32)
    for b in range(B):
        out[b] = parts[2 * b] + parts[2 * b + 1] + b_proj[None, :]
    return out
